# revision 1
# baseline (speedup 1.0000x reference)
"""Trainium2 Bass kernel for nn_ChunkwiseRecurrentAttentionCell.

Math (per (b,h) slice; T=256, Dk=Dv=128):
    gc = cumsum(g);  A = tril(beta_i exp(gc_i-gc_j) k_i.k_j, -1)
    v_new = (I+A)^{-1} (beta v - beta exp(gc) (k @ S0))
    out   = exp(gc) (q@S0) + (tril(exp(gc_i-gc_j),0) * (q k^T)) @ v_new
    S_new = exp(gc_T) S0 + k^T (v_new * exp(gc_T - gc))

Implemented as a chunked recurrence (2 chunks of 128) so all per-chunk exp
ratios are bounded by e^6.4 (fp16-safe).  The triangular solve uses an
8-term Neumann product form  (I+X^4)(I+X^2)(I+X), X = -A_chunk, with dual
power chains (both X^p and its transpose built by matmuls from masked
scalings of the symmetric K K^T — no big transposes needed).  All matmul
operands are fp16 (PE runs fp16 at 1 cycle/row vs fp32's 4); accumulation
is fp32 in PSUM.  Relative error vs the fp32 reference ~ 4e-4.

Sharding: (B,H) flattened to 512 independent slices, 64 per core across
8 NeuronCores (data parallel, no collectives).
"""

import os
import numpy as np

import concourse.bass as bass
import concourse.mybir as mybir
from concourse import bacc
from concourse.tile import TileContext
from concourse.masks import (
    make_identity,
    make_lower_triangular,
    make_upper_triangular,
)

B, H, T, DK, DV = 16, 32, 256, 128, 128
N_CORES = 8
N_SLICES = (B * H) // N_CORES  # 64 per core
CH = 128  # chunk length
N_CHUNKS = T // CH
LEVELS = 3  # Neumann product-form levels -> 2^3 = 8 series terms

F32 = mybir.dt.float32
MM_DT = mybir.dt.float16

_ALU = mybir.AluOpType
_ACTF = mybir.ActivationFunctionType


def build_nc(n_slices: int = N_SLICES):
    nc = bacc.Bacc("TRN2", target_bir_lowering=False)

    dq = nc.dram_tensor("q", [n_slices, T, DK], F32, kind="ExternalInput")
    dk = nc.dram_tensor("k", [n_slices, T, DK], F32, kind="ExternalInput")
    dv = nc.dram_tensor("v", [n_slices, T, DV], F32, kind="ExternalInput")
    dg = nc.dram_tensor("g", [n_slices, T], F32, kind="ExternalInput")
    db = nc.dram_tensor("beta", [n_slices, T], F32, kind="ExternalInput")
    ds0 = nc.dram_tensor("s0", [n_slices, DK, DV], F32, kind="ExternalInput")
    dout = nc.dram_tensor("out", [n_slices, T, DV], F32, kind="ExternalOutput")
    dsn = nc.dram_tensor("s_new", [n_slices, DK, DV], F32, kind="ExternalOutput")

    with TileContext(nc) as tc:
        with (
            tc.tile_pool(name="const", bufs=1) as cpool,
            tc.tile_pool(name="io", bufs=3) as iop,
            tc.tile_pool(name="ops", bufs=3) as opp,
            tc.tile_pool(name="state", bufs=2) as stp,
            tc.tile_pool(name="ps", bufs=1, space="PSUM") as psp,
        ):
            # ---------------- constants ----------------
            ident16 = cpool.tile([128, 128], MM_DT)
            make_identity(nc, ident16)
            ident32 = cpool.tile([128, 128], F32)
            make_identity(nc, ident32)
            mask_sl = cpool.tile([128, 128], F32)  # strict lower ones
            make_lower_triangular(nc, mask_sl, val=1.0, diag=False)
            mask_su = cpool.tile([128, 128], F32)  # strict upper ones
            make_upper_triangular(nc, mask_su, val=1.0, diag=False)
            mask_ui = cpool.tile([128, 128], F32)  # upper ones incl diag
            make_upper_triangular(nc, mask_ui, val=1.0, diag=True)

            # ---------------- per-core setup: gate vectors ----------------
            gt = cpool.tile([n_slices, T], F32)
            nc.sync.dma_start(gt[:], dg[:])
            bt = cpool.tile([n_slices, T], F32)
            nc.sync.dma_start(bt[:], db[:])
            gct = cpool.tile([n_slices, T], F32)
            nc.vector.tensor_tensor_scan(
                gct[:], gt[:], gt[:], 0.0, op0=_ALU.add, op1=_ALU.bypass
            )
            gcl1 = cpool.tile([n_slices, CH], F32)
            nc.vector.tensor_scalar(
                gcl1[:], gct[:, CH : 2 * CH], gct[:, CH - 1 : CH], None,
                op0=_ALU.subtract,
            )

            # per chunk: r, 1/r, -beta*r  in [n_slices, CH]; then transpose to
            # [CH, n_slices] so columns are per-slice partition-scalars.
            rT, irT, nbrT, bT, ET = [], [], [], [], []
            for c in range(N_CHUNKS):
                gcl = gct[:, 0:CH] if c == 0 else gcl1[:]
                r_c = cpool.tile([n_slices, CH], F32, name=f"r_{c}")
                nc.scalar.activation(r_c[:], gcl, _ACTF.Exp)
                ir_c = cpool.tile([n_slices, CH], F32, name=f"ir_{c}")
                nc.scalar.activation(ir_c[:], gcl, _ACTF.Exp, scale=-1.0)
                nbr_c = cpool.tile([n_slices, CH], F32, name=f"nbr_{c}")
                nc.vector.scalar_tensor_tensor(
                    nbr_c[:],
                    bt[:, c * CH : (c + 1) * CH],
                    -1.0,
                    r_c[:],
                    op0=_ALU.mult,
                    op1=_ALU.mult,
                )
                outs = []
                for src, nm in (
                    (r_c[:], "rT"),
                    (ir_c[:], "irT"),
                    (nbr_c[:], "nbrT"),
                    (bt[:, c * CH : (c + 1) * CH], "bT"),
                ):
                    pst = psp.tile([CH, n_slices], F32, name=f"pst_{nm}{c}", tag="ps_t", bufs=3)
                    nc.tensor.transpose(pst[:], src, ident32[0:n_slices, 0:n_slices])
                    dst = cpool.tile([CH, n_slices], F32, name=f"{nm}_{c}")
                    nc.scalar.copy(dst[:], pst[:])
                    outs.append(dst)
                rT.append(outs[0])
                irT.append(outs[1])
                nbrT.append(outs[2])
                bT.append(outs[3])
                ps_e = psp.tile([1, n_slices], F32, name=f"ps_e{c}", tag="ps_t", bufs=3)
                nc.tensor.transpose(
                    ps_e[:], r_c[:, CH - 1 : CH], ident32[0:n_slices, 0:n_slices]
                )
                e_row = cpool.tile([1, n_slices], F32, name=f"e_row_{c}")
                nc.scalar.copy(e_row[:], ps_e[:])
                e_c = cpool.tile([CH, n_slices], F32, name=f"ET_{c}")
                nc.gpsimd.partition_broadcast(e_c[:], e_row[0:1, :])
                ET.append(e_c)

            # ---------------- main loop over slices ----------------
            for s in range(n_slices):
                s_cur = None
                for c in range(N_CHUNKS):
                    tsl = slice(c * CH, (c + 1) * CH)
                    q_c = iop.tile([CH, DK], F32, name="q_c")
                    nc.sync.dma_start(q_c[:], dq[s, tsl, :])
                    k_c = iop.tile([CH, DK], F32, name="k_c")
                    nc.sync.dma_start(k_c[:], dk[s, tsl, :])
                    v_c = iop.tile([CH, DV], F32, name="v_c")
                    nc.sync.dma_start(v_c[:], dv[s, tsl, :])
                    if c == 0:
                        s_f32 = iop.tile([DK, DV], F32, name="s_f32")
                        nc.sync.dma_start(s_f32[:], ds0[s, :, :])
                        s_cur = stp.tile([DK, DV], MM_DT, name="s_cur")
                        nc.gpsimd.tensor_copy(s_cur[:], s_f32[:])

                    # scaled copies (fp16)
                    qr = opp.tile([CH, DK], MM_DT, name="qr")
                    nc.scalar.activation(
                        qr[:], q_c[:], _ACTF.Copy, scale=rT[c][:, s : s + 1]
                    )
                    knbr = opp.tile([CH, DK], MM_DT, name="knbr")
                    nc.vector.tensor_scalar_mul(knbr[:], k_c[:], nbrT[c][:, s : s + 1])
                    kir = opp.tile([CH, DK], MM_DT, name="kir")
                    nc.vector.tensor_scalar_mul(kir[:], k_c[:], irT[c][:, s : s + 1])

                    # transposes (PE) + copies (ACT)
                    qT = opp.tile([DK, CH], MM_DT, name="qT")
                    kTn = opp.tile([DK, CH], MM_DT, name="kTn")
                    kTi = opp.tile([DK, CH], MM_DT, name="kTi")
                    for src, dst, nm in ((qr, qT, "q"), (knbr, kTn, "n"), (kir, kTi, "i")):
                        ps_t = psp.tile([DK, CH], MM_DT, name=f"ps_t{nm}", tag="ps_t", bufs=3)
                        nc.tensor.transpose(ps_t[:], src[:], ident16[:])
                        nc.scalar.copy(dst[:], ps_t[:])

                    # Y = beta*v + (knbr @ S)     [= beta*v - beta*r*(k@S)]
                    ps_y = psp.tile([CH, DV], F32, name="ps_y", tag="mm", bufs=3)
                    nc.tensor.matmul(ps_y[:], kTn[:], s_cur[:])
                    z = opp.tile([CH, DV], MM_DT, name="z_it", tag="z", bufs=4)
                    nc.vector.scalar_tensor_tensor(
                        z[:], v_c[:], bT[c][:, s : s + 1], ps_y[:],
                        op0=_ALU.mult, op1=_ALU.add,
                    )

                    # B0 = -A = strict_tril(knbr @ kir^T); C0 = B0^T
                    ps_a = psp.tile([CH, CH], F32, name="ps_a", tag="mm", bufs=3)
                    nc.tensor.matmul(ps_a[:], kTn[:], kTi[:])
                    b0 = opp.tile([CH, CH], MM_DT, name="b0")
                    nc.vector.tensor_tensor(b0[:], ps_a[:], mask_sl[:], _ALU.mult)
                    ps_at = psp.tile([CH, CH], F32, name="ps_at", tag="mm", bufs=3)
                    nc.tensor.matmul(ps_at[:], kTi[:], kTn[:])
                    c0 = opp.tile([CH, CH], MM_DT, name="c0")
                    nc.vector.tensor_tensor(c0[:], ps_at[:], mask_su[:], _ALU.mult)

                    # dual chain: B1 = B0@B0, C1 = C0@C0, C2 = C1@C1
                    ps_b1 = psp.tile([CH, CH], F32, name="ps_b1", tag="mm", bufs=3)
                    nc.tensor.matmul(ps_b1[:], c0[:], b0[:])
                    b1 = opp.tile([CH, CH], MM_DT, name="b1")
                    nc.scalar.copy(b1[:], ps_b1[:])
                    ps_c1 = psp.tile([CH, CH], F32, name="ps_c1", tag="mm", bufs=3)
                    nc.tensor.matmul(ps_c1[:], b0[:], c0[:])
                    c1 = opp.tile([CH, CH], MM_DT, name="c1")
                    nc.scalar.copy(c1[:], ps_c1[:])
                    ps_c2 = psp.tile([CH, CH], F32, name="ps_c2", tag="mm", bufs=3)
                    nc.tensor.matmul(ps_c2[:], b1[:], c1[:])
                    c2 = opp.tile([CH, CH], MM_DT, name="c2")
                    nc.vector.tensor_copy(c2[:], ps_c2[:])

                    # applies: z <- z + X^(2^j) z   (lhsT = C_j)
                    for cj in (c0, c1, c2):
                        ps_ap = psp.tile([CH, DV], F32, name="ps_ap", tag="mm", bufs=3)
                        nc.tensor.matmul(ps_ap[:], cj[:], z[:])
                        z_new = opp.tile([CH, DV], MM_DT, name="z_new", tag="z", bufs=4)
                        nc.vector.tensor_tensor(z_new[:], ps_ap[:], z[:], _ALU.add)
                        z = z_new

                    # CQT = triu(kir @ qr^T, 0)
                    ps_cq = psp.tile([CH, CH], F32, name="ps_cq", tag="mm", bufs=3)
                    nc.tensor.matmul(ps_cq[:], kTi[:], qT[:])
                    cqt = opp.tile([CH, CH], MM_DT, name="cqt")
                    nc.vector.tensor_tensor(cqt[:], ps_cq[:], mask_ui[:], _ALU.mult)

                    # out = qr @ S + CQT^T @ z
                    ps_o = psp.tile([CH, DV], F32, name="ps_o", tag="ps_o", bufs=1)
                    nc.tensor.matmul(ps_o[:], qT[:], s_cur[:], start=True, stop=False)
                    nc.tensor.matmul(ps_o[:], cqt[:], z[:], start=False, stop=True)
                    o_sb = opp.tile([CH, DV], F32, name="o_sb")
                    nc.scalar.copy(o_sb[:], ps_o[:])
                    nc.sync.dma_start(dout[s, tsl, :], o_sb[:])

                    # state update: S' = E*(S + kir^T @ z)  [folded: Zs = E*z]
                    zs = opp.tile([CH, DV], MM_DT, name="zs")
                    nc.scalar.activation(
                        zs[:], z[:], _ACTF.Copy, scale=ET[c][:, s : s + 1]
                    )
                    ps_s = psp.tile([DK, DV], F32, name="ps_s", tag="ps_s", bufs=1)
                    nc.tensor.matmul(ps_s[:], kir[:], zs[:])
                    if c < N_CHUNKS - 1:
                        s_next = stp.tile([DK, DV], MM_DT, name="s_next")
                        nc.vector.scalar_tensor_tensor(
                            s_next[:], s_cur[:], ET[c][:, s : s + 1], ps_s[:],
                            op0=_ALU.mult, op1=_ALU.add,
                        )
                        s_cur = s_next
                    else:
                        s_fin = stp.tile([DK, DV], F32, name="s_fin")
                        nc.vector.scalar_tensor_tensor(
                            s_fin[:], s_cur[:], ET[c][:, s : s + 1], ps_s[:],
                            op0=_ALU.mult, op1=_ALU.add,
                        )
                        nc.sync.dma_start(dsn[s, :, :], s_fin[:])

    nc.compile()
    return nc


_NC_CACHE = {}


def _get_nc(n_slices):
    if n_slices not in _NC_CACHE:
        _NC_CACHE[n_slices] = build_nc(n_slices)
    return _NC_CACHE[n_slices]


def kernel(q, k, v, g, beta, last_recurrent_state):
    from concourse.bass_utils import run_bass_kernel_spmd

    qf = np.ascontiguousarray(q, np.float32).reshape(B * H, T, DK)
    kf = np.ascontiguousarray(k, np.float32).reshape(B * H, T, DK)
    vf = np.ascontiguousarray(v, np.float32).reshape(B * H, T, DV)
    gf = np.ascontiguousarray(g, np.float32).reshape(B * H, T)
    bf = np.ascontiguousarray(beta, np.float32).reshape(B * H, T)
    sf = np.ascontiguousarray(last_recurrent_state, np.float32).reshape(B * H, DK, DV)

    nc = _get_nc(N_SLICES)
    in_maps = []
    for i in range(N_CORES):
        sl = slice(i * N_SLICES, (i + 1) * N_SLICES)
        in_maps.append(
            {
                "q": qf[sl],
                "k": kf[sl],
                "v": vf[sl],
                "g": gf[sl],
                "beta": bf[sl],
                "s0": sf[sl],
            }
        )
    res = run_bass_kernel_spmd(nc, in_maps, list(range(N_CORES)))
    out = np.concatenate([res.results[i]["out"] for i in range(N_CORES)], axis=0)
    s_new = np.concatenate([res.results[i]["s_new"] for i in range(N_CORES)], axis=0)
    return np.concatenate([out.reshape(-1), s_new.reshape(-1)], axis=0)



# revision 18
# speedup vs baseline: 1.4918x; 1.4918x over previous
"""Trainium2 Bass kernel for nn_ChunkwiseRecurrentAttentionCell.

Math (per (b,h) slice; T=256, Dk=Dv=128, 2 chunks of CH=128):
    gc = cumsum(g);  X = -tril(beta_i exp(gc_i-gc_j) k_i.k_j, -1)
    v_new = (I+X+X^2+X^3) (beta v - beta exp(gc) (k @ S))      [Horner]
    out   = exp(gc) (q@S) + (tril(exp(gc_i-gc_j),0) * (q k^T)) @ v_new
    S'    = exp(gc_T) S + k^T (v_new * exp(gc_T - gc))

Design notes (v2, instruction-count-minimized):
  - Host precomputes fp16 inputs + per-(slice,chunk) scale columns
    (nbr=-beta*r, ir=1/r, r, E=r[-1]) and pre-scaled tensors
    vhat=beta*v, khat=k*ir*E, masknbr[a,b]=triu(1,)*nbr_b.  This kills
    the whole device preamble and several per-chunk elementwise ops.
  - k^T / q^T tiles come straight from DRAM via the XBAR transpose DMA
    (fp16) - no PE transposes, no PSUM round-trips.
  - One wide matmul kT-stationary x [kT|qT] yields K K^T and k q^T
    together; K K^T is symmetric so a single product serves both the
    X (row-scaled) and X^T (col-scaled) roles.
  - Neumann solve evaluated in Horner form with c0 = X^T as the only
    stationary: z3 = z0 + X(z0 + X(z0 + X z0)).
  - All DMA is batched into a handful of giant strided transfers per
    slice-group (HWDGE issue cost ~625ns each dominates small DMAs).
  - Engine balance: PE ~8 matmuls/chunk, DVE 5 ops, Pool (gpsimd,
    no PSUM port) 2 SBUF-side mask ops, ACT 2 (wide PSUM copy + out).

Sharding: (B,H) flattened to 512 independent slices, 64 per core across
8 NeuronCores (data parallel, no collectives).
"""

import numpy as np

import concourse.bass as bass
import concourse.mybir as mybir
from concourse import bacc
from concourse.tile import TileContext

B, H, T, DK, DV = 16, 32, 256, 128, 128
N_CORES = 8
N_SLICES = (B * H) // N_CORES  # 64 per core
CH = 128
NCH = T // CH  # 2
GROUP = 16  # slices per DMA group

F32 = mybir.dt.float32
F16 = mybir.dt.float16

_ALU = mybir.AluOpType
_ACTF = mybir.ActivationFunctionType

# scale-pack column kinds
K_NBR, K_IR, K_R, K_E = 0, 1, 2, 3


def build_nc(n_slices: int = N_SLICES, group: int = GROUP):
    assert n_slices % group == 0
    ngroups = n_slices // group
    nc = bacc.Bacc("TRN2", target_bir_lowering=False)

    d_q = nc.dram_tensor("q", [n_slices * T, DK], F16, kind="ExternalInput")
    d_k = nc.dram_tensor("k", [n_slices * T, DK], F16, kind="ExternalInput")
    d_vh = nc.dram_tensor("vh", [CH, n_slices * NCH * DV], F16, kind="ExternalInput")
    d_kh = nc.dram_tensor("kh", [CH, n_slices * NCH * DK], F16, kind="ExternalInput")
    d_mn = nc.dram_tensor("mn", [CH, n_slices * NCH * CH], F16, kind="ExternalInput")
    d_mi = nc.dram_tensor("mi", [CH, n_slices * NCH * CH], F16, kind="ExternalInput")
    d_s0 = nc.dram_tensor("s0", [DK, n_slices * DV], F16, kind="ExternalInput")
    d_sc = nc.dram_tensor("sc", [CH, 4 * n_slices * NCH], F32, kind="ExternalInput")
    d_out = nc.dram_tensor("out", [CH, n_slices * NCH * DV], F16, kind="ExternalOutput")
    d_sn = nc.dram_tensor("s_new", [DK, n_slices * DV], F16, kind="ExternalOutput")

    with TileContext(nc) as tc:
        with (
            tc.tile_pool(name="const", bufs=1) as cp,
            tc.tile_pool(name="mega", bufs=2) as mp,
            tc.tile_pool(name="small", bufs=6) as sp,
            tc.tile_pool(name="state", bufs=3) as stp,
            tc.tile_pool(name="ps", bufs=1, space="PSUM") as pp,
        ):
            sc = cp.tile([CH, 4 * n_slices * NCH], F32)
            nc.sync.dma_start(sc[:], d_sc[:])

            def col(kind, s, c):
                i = kind * (n_slices * NCH) + s * NCH + c
                return sc[:, i : i + 1]

            for grp in range(ngroups):
                g0 = grp * group
                gt = slice(g0 * T, (g0 + group) * T)
                gB = slice(g0 * NCH * CH, (g0 + group) * NCH * CH)
                gS = slice(g0 * DV, (g0 + group) * DV)

                kTm = mp.tile([DK, group * NCH * CH], F16, name="kTm", tag="kTm")
                nc.sync.dma_start(kTm[:], d_k[gt, :], transpose=True)
                qTm = mp.tile([DK, group * NCH * CH], F16, name="qTm", tag="qTm")
                nc.sync.dma_start(qTm[:], d_q[gt, :], transpose=True)
                vh = mp.tile([CH, group * NCH * DV], F16, name="vh", tag="vh")
                nc.sync.dma_start(vh[:], d_vh[:, gB])
                kh = mp.tile([CH, group * NCH * DK], F16, name="kh", tag="kh")
                nc.sync.dma_start(kh[:], d_kh[:, gB])
                mn = mp.tile([CH, group * NCH * CH], F16, name="mn", tag="mn")
                nc.sync.dma_start(mn[:], d_mn[:, gB])
                mi = mp.tile([CH, group * NCH * CH], F16, name="mi", tag="mi")
                nc.sync.dma_start(mi[:], d_mi[:, gB])
                s0t = mp.tile([DK, group * DV], F16, name="s0t", tag="s0t")
                nc.sync.dma_start(s0t[:], d_s0[:, gS])
                outst = mp.tile([CH, group * NCH * DV], F16, name="outst", tag="outst")
                snst = mp.tile([DK, group * DV], F16, name="snst", tag="snst")

                for si in range(group):
                    s = g0 + si
                    s_cur = s0t[:, si * DV : (si + 1) * DV]
                    for c in range(NCH):
                        blk = si * NCH + c
                        bsl = slice(blk * CH, (blk + 1) * CH)
                        kT = kTm[:, bsl]
                        qT = qTm[:, bsl]

                        # K K^T, k q^T, k@S: one stationary (kT), one PSUM bank
                        ps_big = pp.tile(
                            [CH, 3 * CH], F32, name="ps_big", tag="ps_big", bufs=3
                        )
                        nc.tensor.matmul(ps_big[:, 0:CH], kT, kT)
                        nc.tensor.matmul(ps_big[:, CH : 2 * CH], kT, qT)
                        nc.tensor.matmul(ps_big[:, 2 * CH : 3 * CH], kT, s_cur)
                        sb2 = sp.tile([CH, 2 * CH], F16, name="sb2", tag="sb2", bufs=4)
                        nc.scalar.copy(sb2[:], ps_big[:, 0 : 2 * CH])

                        # z0 = beta*v - beta*r*(k@S) = nbr*(k@S) + vhat
                        z0 = sp.tile([CH, DV], F16, name="z0", tag="z0")
                        nc.vector.scalar_tensor_tensor(
                            z0[:], ps_big[:, 2 * CH : 3 * CH], col(K_NBR, s, c),
                            vh[:, bsl], op0=_ALU.mult, op1=_ALU.add,
                        )

                        # c0 = X^T = G[a,b] * (triu1 * nbr_b * ir_a)[a,b]
                        c0 = sp.tile([CH, CH], F16, name="c0", tag="c0")
                        nc.gpsimd.tensor_tensor(
                            c0[:], sb2[:, 0:CH], mn[:, bsl], _ALU.mult
                        )
                        # cqt[a,b] = (k_a.q_b) * (triu0 * ir_a)[a,b]
                        cqt = sp.tile([CH, CH], F16, name="cqt", tag="cqt")
                        nc.gpsimd.tensor_tensor(
                            cqt[:], sb2[:, CH : 2 * CH], mi[:, bsl], _ALU.mult
                        )

                        # Horner: z3 = z0 + X(z0 + X(z0 + X z0)); X@u = c0^T@u
                        u = z0
                        for j in range(3):
                            ps_h = pp.tile(
                                [CH, DV], F32, name="ps_h", tag="ps_h", bufs=3
                            )
                            nc.tensor.matmul(ps_h[:], c0[:], u[:])
                            un = sp.tile([CH, DV], F16, name=f"u{j}", tag=f"u{j}")
                            nc.vector.tensor_tensor(un[:], ps_h[:], z0[:], _ALU.add)
                            u = un

                        # out = r * (q@S + cqt^T @ z3); state matmul shares the bank
                        ps_os = pp.tile(
                            [CH, 2 * DV], F32, name="ps_os", tag="ps_os", bufs=2
                        )
                        nc.tensor.matmul(
                            ps_os[:, 0:DV], qT, s_cur, start=True, stop=False
                        )
                        nc.tensor.matmul(
                            ps_os[:, 0:DV], cqt[:], u[:], start=False, stop=True
                        )
                        nc.scalar.activation(
                            outst[:, blk * DV : (blk + 1) * DV], ps_os[:, 0:DV],
                            _ACTF.Copy, scale=col(K_R, s, c),
                        )

                        # state: S' = E*S + khat^T @ z3
                        nc.tensor.matmul(
                            ps_os[:, DV : 2 * DV],
                            kh[:, blk * DK : (blk + 1) * DK], u[:],
                        )
                        if c < NCH - 1:
                            s_nx = stp.tile([DK, DV], F16, name="s_nx", tag="s_nx")
                            dst = s_nx[:]
                        else:
                            dst = snst[:, si * DV : (si + 1) * DV]
                        nc.vector.scalar_tensor_tensor(
                            dst, s_cur, col(K_E, s, c), ps_os[:, DV : 2 * DV],
                            op0=_ALU.mult, op1=_ALU.add,
                        )
                        if c < NCH - 1:
                            s_cur = dst

                nc.sync.dma_start(d_out[:, gB], outst[:])
                nc.sync.dma_start(d_sn[:, gS], snst[:])

    nc.compile()
    return nc


_NC_CACHE = {}


def _get_nc(n_slices=N_SLICES, group=GROUP):
    key = (n_slices, group)
    if key not in _NC_CACHE:
        _NC_CACHE[key] = build_nc(n_slices, group)
    return _NC_CACHE[key]


def _host_prep(q, k, v, g, beta, S0, n_slices, n_cores=N_CORES):
    """Per-core input prep: fp16 conversion, scale columns, pre-scaled
    tensors, partition-major DRAM layouts."""
    f16 = np.float16
    NSall = q.shape[0]
    gc = np.cumsum(g, axis=1)  # [NS, T] f32
    gcl = gc.reshape(NSall, NCH, CH).copy()
    for c in range(1, NCH):
        gcl[:, c, :] -= gc[:, c * CH - 1][:, None]
    r = np.exp(gcl)
    ir = np.exp(-gcl)
    E = r[:, :, -1]  # [NS, NCH]
    b2 = beta.reshape(NSall, NCH, CH)
    nbr = -b2 * r
    irE = ir * E[:, :, None]

    q16 = q.astype(f16)
    k16 = k.astype(f16)
    vhat = (beta[:, :, None] * v).astype(f16).reshape(NSall, NCH, CH, DV)
    khat = (k.reshape(NSall, NCH, CH, DK) * irE[..., None]).astype(f16)
    mask_su = np.triu(np.ones((CH, CH), np.float32), 1)
    mask_ui = np.triu(np.ones((CH, CH), np.float32), 0)
    # ir_a on the partition axis folded into the host mask tiles
    masknbr = (
        mask_su[None, None] * nbr[:, :, None, :] * ir[:, :, :, None]
    ).astype(f16)
    maskir = (mask_ui[None, None] * ir[:, :, :, None]).astype(f16)
    s016 = S0.astype(f16)

    def pm(x):  # [ns, NCH, CH, D] -> [CH, ns*NCH*D] partition-major
        return np.ascontiguousarray(
            x.transpose(2, 0, 1, 3).reshape(CH, -1)
        )

    maps = []
    for i in range(n_cores):
        sl = slice(i * n_slices, (i + 1) * n_slices)
        sc = np.concatenate(
            [
                np.ascontiguousarray(nbr[sl].transpose(2, 0, 1).reshape(CH, -1)),
                np.ascontiguousarray(ir[sl].transpose(2, 0, 1).reshape(CH, -1)),
                np.ascontiguousarray(r[sl].transpose(2, 0, 1).reshape(CH, -1)),
                np.ascontiguousarray(
                    np.broadcast_to(
                        E[sl][None, :, :], (CH, n_slices, NCH)
                    ).reshape(CH, -1)
                ),
            ],
            axis=1,
        ).astype(np.float32)
        maps.append(
            {
                "q": q16[sl].reshape(n_slices * T, DK),
                "k": k16[sl].reshape(n_slices * T, DK),
                "vh": pm(vhat[sl]),
                "kh": pm(khat[sl]),
                "mn": pm(masknbr[sl]),
                "mi": pm(maskir[sl]),
                "s0": np.ascontiguousarray(
                    s016[sl].transpose(1, 0, 2).reshape(DK, -1)
                ),
                "sc": sc,
            }
        )
    return maps


def kernel(q, k, v, g, beta, last_recurrent_state):
    from concourse.bass_utils import run_bass_kernel_spmd

    qf = np.ascontiguousarray(q, np.float32).reshape(B * H, T, DK)
    kf = np.ascontiguousarray(k, np.float32).reshape(B * H, T, DK)
    vf = np.ascontiguousarray(v, np.float32).reshape(B * H, T, DV)
    gf = np.ascontiguousarray(g, np.float32).reshape(B * H, T)
    bf = np.ascontiguousarray(beta, np.float32).reshape(B * H, T)
    sf = np.ascontiguousarray(last_recurrent_state, np.float32).reshape(
        B * H, DK, DV
    )

    nc = _get_nc()
    in_maps = _host_prep(qf, kf, vf, gf, bf, sf, N_SLICES)
    res = run_bass_kernel_spmd(nc, in_maps, list(range(N_CORES)))

    out = np.empty((B * H, T, DV), np.float32)
    s_new = np.empty((B * H, DK, DV), np.float32)
    for i in range(N_CORES):
        sl = slice(i * N_SLICES, (i + 1) * N_SLICES)
        o = res.results[i]["out"]  # [CH, NS*NCH*DV] f16
        out[sl] = (
            o.reshape(CH, N_SLICES, NCH, DV)
            .transpose(1, 2, 0, 3)
            .reshape(N_SLICES, T, DV)
            .astype(np.float32)
        )
        sn = res.results[i]["s_new"]  # [DK, NS*DV] f16
        s_new[sl] = (
            sn.reshape(DK, N_SLICES, DV).transpose(1, 0, 2).astype(np.float32)
        )
    return np.concatenate([out.reshape(-1), s_new.reshape(-1)], axis=0)
